# revision 51
# baseline (speedup 1.0000x reference)
"""AttentiveProtoFusion kernel for 8 TRN2 NeuronCores (v3).

Math (identical algebra to the reference, ~14x fewer FLOPs):
    qp = sent @ (Wq @ Wk^T) + bq @ Wk^T              [n, 768]
    scores[n,p] = sum_c proto[n,p,c] * qp[n,c]   (+ qp.bk const -> dropped)
    w = softmax(scores, axis=p)
    ctx[n,c] = sum_p w[n,p] * proto[n,p,c]

Design (HW-measured per-op costs in brackets; ~87us vs 164us baseline):
  * proto / sent / W staged host-side in 16-bit (same values, half the
    DMA: ~15.5 MB/core -> ~41us roofline). 20 protos fp16 + 12 protos
    bf16 (full-bf16 scores would be 1.6e-2 rel err vs the 2e-2 gate;
    this mix measures 9.1e-3 on HW).
  * Pooling U[n,c] += e[n,p]*T_p[n,c] runs on the TensorEngine as a
    matmul with DIAGONAL stationary diag(e[:,p]) (bf16, built by one
    tensor_scalar [186ns] or ACT copy-scale [450ns] from a staged
    identity), PSUM-accumulated over all 32 protos [163ns matmul +
    97ns ldweights per half]. out[i,j] = sum_k diag[k,i]*T[k,j] =
    e[i]*T[i,j]. This moves the entire weighted-pooling phase off the
    vector engines onto the otherwise-idle PE.
  * Scores are inherently batched per-token dots (no matmul form - the
    per-token operand would have to ride the stationary side). fp16
    protos: one DVE affine_mul_reduce each [873ns; in1 MUST be f32 -
    fp16 in1 measures 1412ns; fp16 has no 2x DVE uops]. bf16 protos:
    DVE tensor_tensor product in the bf16-only 2x mode [474ns] + ACT
    accum-reduce [605+185ns]. GPSIMD is left idle: its tensor_tensor is
    2.4ns/elem AND its shared SBUF port stretches concurrent DVE ops.
  * Proto tiles carry BOTH 128-token blocks of a proto pair
    ([128, 2, 2, 768] strided DMA), so block-1 work is not serialized
    behind block 0. Weight DMAs stream alone first (proto stream held
    ~7us): outstanding DMAs round-robin per-packet, and qp gates all
    scores. Per-dd sent/W slices let projection matmuls start early.
  * Online softmax with fixed exponent frame from chunk 0 (Mhat =
    max(chunk0)+60, scores clamped at Mhat+80): exp(s-Mhat) <= e^80
    fits f32/bf16; U/Z equals softmax exactly. Chunks of [12,12,6,2]
    protos - the tiny last chunk shrinks the critical-path tail.
  * Chunk c's diag/MAC ops are emitted AFTER chunk c+1's scores: engine
    queues are in-order, so a DVE diag waiting on ACT's exp would
    head-of-line-block the next chunk's score ops.

Sharding: data-parallel over the 2048 tokens (B*S), 256 tokens/core.
"""

import sys

for _p in ("/opt/trn_rl_repo", "/opt/pypackages"):
    if _p not in sys.path:
        sys.path.append(_p)

import numpy as np

B, S, P, D_SENT, D_CTX = 4, 512, 32, 1024, 768
N_CORES = 8
TOK = B * S                    # 2048
TPC = TOK // N_CORES           # 256 tokens per core
BLK = 128                      # tokens per block
NBLK = TPC // BLK              # 2
PG = 2                         # prototypes per DMA tile
NPG = P // PG                  # 16 proto tiles (each carries both blocks)
CHUNK_TILES = [6, 6, 3, 1]     # tiles per online chunk -> protos [12,12,6,2]
NCH = len(CHUNK_TILES)
EH = D_CTX // 2                # 384 (psum-bank-sized half of d_ctx)
DS = D_SENT // 128             # 8 contraction chunks for the projection

# Proto pairs staged in bf16 instead of fp16: their score products run as
# DVE tensor_tensor in the bf16 2x mode (474ns vs amr's 873ns) with the
# 768-wide sum on ACT (Copy accum_out). Full-bf16 scores would be 1.6e-2
# rel err (too close to the 2e-2 gate); this 12-proto subset sims at
# 7.0e-3 (~1.0e-2 expected on HW, 2x margin).
BF_PAIRS = {0, 2, 4, 6, 8, 10}          # pair g covers protos 2g, 2g+1
BF_P = {2 * g + j for g in BF_PAIRS for j in range(2)}
# diag(e_p) builder: ACT copy-scale (~450ns) for early protos keeps DVE
# free while scores stream; later protos on DVE (186ns) - by then DVE is
# drained and the chunk tail is latency-critical.
DIAG_ON_ACT = set(range(20))

_NC = None


def _build():
    import concourse.bass as bass
    import concourse.tile as tile
    from concourse import bacc, mybir

    f32 = mybir.dt.float32
    f16 = mybir.dt.float16
    bf16 = mybir.dt.bfloat16
    Alu = mybir.AluOpType
    Act = mybir.ActivationFunctionType
    X = mybir.AxisListType.X

    nc = bacc.Bacc("TRN2", target_bir_lowering=False)

    # sentT/w staged host-side ALREADY in on-chip layout [partition, dd, :]
    # so the DMAs are contiguous >=1.5KB per-partition lines (the rearranged
    # AP forms produced 512B descriptors and ran at ~244 GB/s)
    n_bf = 2 * len(BF_PAIRS)
    sentT_d = nc.dram_tensor("sentT", [128, DS, TPC], f16, kind="ExternalInput")
    proto_d = nc.dram_tensor(
        "proto", [TPC, P - n_bf, D_CTX], f16, kind="ExternalInput")
    protobf_d = nc.dram_tensor(
        "protobf", [TPC, n_bf, D_CTX], bf16, kind="ExternalInput")
    w_d = nc.dram_tensor("w", [128, DS, D_CTX], f16, kind="ExternalInput")
    bp_d = nc.dram_tensor("bp", [1, D_CTX], f16, kind="ExternalInput")
    eye_d = nc.dram_tensor("eye", [BLK, BLK], bf16, kind="ExternalInput")
    out_d = nc.dram_tensor("out", [TPC, D_CTX], f32, kind="ExternalOutput")

    with tile.TileContext(nc) as tc:
        with (
            tc.tile_pool(name="wpool", bufs=1) as wpool,
            tc.tile_pool(name="persist", bufs=1) as persist,
            tc.tile_pool(name="ppool", bufs=NPG - len(BF_PAIRS)) as ppool,
            tc.tile_pool(name="ppoolb", bufs=len(BF_PAIRS)) as ppoolb,
            tc.tile_pool(name="prodp", bufs=6) as prodp,
            tc.tile_pool(name="diagp", bufs=8) as diagp,
            tc.tile_pool(name="small", bufs=4) as small,
            tc.tile_pool(name="psq", bufs=2, space="PSUM") as psq,
            tc.tile_pool(name="psu", bufs=1, space="PSUM") as psu,
        ):
            # ---- weights / staged constants --------------------------------
            # One DMA per contraction chunk so the projection matmuls start
            # as soon as their slice lands instead of after the whole 2.1 MB.
            bp_sb = wpool.tile([1, D_CTX], f16)
            nc.sync.dma_start(out=bp_sb[:], in_=bp_d[:])
            eye_sb = wpool.tile([128, BLK], bf16)
            nc.sync.dma_start(out=eye_sb[:], in_=eye_d[:])
            sentT_sb = wpool.tile([128, DS, TPC], f16)
            w_sb = wpool.tile([128, DS, D_CTX], f16)
            # interleave per-dd sent/W slices so the first projection matmul
            # can fire after ~130 KB instead of the whole 2.1 MB
            for dd in range(DS):
                nc.sync.dma_start(out=sentT_sb[:, dd, :], in_=sentT_d[:, dd, :])
                nc.sync.dma_start(out=w_sb[:, dd, :], in_=w_d[:, dd, :])
            ones_sb = wpool.tile([1, 128], f16)
            nc.vector.memset(ones_sb[:], 1.0)

            qp_sb = persist.tile([128, NBLK, D_CTX], f32)
            qpb_sb = persist.tile([128, NBLK, D_CTX], bf16)
            # per-chunk score/exp tiles: separate tiles avoid false
            # WAR/RAW serialization between chunks on one shared tile
            scores = [persist.tile([128, NBLK, cs * PG], f32, name=f"sc{c}")
                      for c, cs in enumerate(CHUNK_TILES)]
            expw = [persist.tile([128, NBLK, cs * PG], f32, name=f"ew{c}")
                    for c, cs in enumerate(CHUNK_TILES)]
            negMhat = persist.tile([128, NBLK, 1], f32)
            clampv = persist.tile([128, NBLK, 1], f32)
            zparts = persist.tile([128, NBLK, NCH], f32)
            outsb = persist.tile([128, NBLK, D_CTX], f32)

            # ---- engine warm-ups (absorb table/uop loads during DMA) -------
            wu16 = wpool.tile([128, 8], f16)
            wu32 = wpool.tile([128, 8], f32)
            wuac = wpool.tile([128, 1], f32)
            nc.vector.memset(wu16[:], 0.0)
            nc.vector.memset(wu32[:], 0.0)
            nc.vector.affine_mul_reduce(
                out=wu16[:], accum_out=wuac[:], in0=wu16[:], in1=wu32[:],
                scale=1.0, bias=0.0,
            )
            # Exp, not Copy: loads the exp_and_others table set (~2.7us)
            # here instead of at chunk 0's first exp mid-stream (Copy is in
            # every set, so later Copies never trigger another load)
            nc.scalar.activation(out=wu32[:], in_=wu32[:], func=Act.Exp)
            # ---- projection: qp = sent @ W + bp  (PE fp16, f32 out) --------
            qp_psums = []
            for b in range(NBLK):
                pps = [psq.tile([128, EH], f32, tag=f"mm{h}", name=f"pp{h}")
                       for h in range(2)]
                for dd in range(DS):
                    for h in range(2):
                        nc.tensor.matmul(
                            pps[h][:],
                            sentT_sb[:, dd, b * BLK:(b + 1) * BLK],
                            w_sb[:, dd, h * EH:(h + 1) * EH],
                            start=(dd == 0),
                            stop=False,
                        )
                for h in range(2):
                    nc.tensor.matmul(
                        pps[h][:],
                        ones_sb[0:1, :],
                        bp_sb[0:1, h * EH:(h + 1) * EH],
                        start=False,
                        stop=True,
                    )
                    nc.scalar.activation(
                        out=qp_sb[:, b, h * EH:(h + 1) * EH], in_=pps[h][:],
                        func=Act.Copy,
                    )
                    nc.scalar.activation(
                        out=qpb_sb[:, b, h * EH:(h + 1) * EH], in_=pps[h][:],
                        func=Act.Copy,
                    )
                qp_psums.append(pps)

            # persistent PSUM accumulators for the pooled context
            U = []
            for b in range(NBLK):
                row = []
                for h in range(2):
                    ubh = psu.tile([128, EH], f32, tag=f"U{b}{h}", name=f"U{b}{h}")
                    row.append(ubh)
                U.append(row)
            # PE warm-up; the data is garbage and overwritten by the real
            # chain's start=True later
            nc.tensor.matmul(
                U[0][0][:, 0:128], ones_sb[0:1, :], ones_sb[0:1, :],
                start=True, stop=True, skip_group_check=True,
            )

            # ---- main loop: stream protos; scores -> exp -> PE pooling -----
            # proto DRAM is [256, 20|12, 768]; a tile carries proto pair g
            # for BOTH 128-token blocks: [n, bb, p, e]
            proto_v = proto_d[:].rearrange("(bb n) p e -> n bb p e", n=BLK)
            protobf_v = protobf_d[:].rearrange("(bb n) p e -> n bb p e", n=BLK)
            # local pair index within each staged tensor
            _lbf, _lfp = {}, {}
            for g in range(NPG):
                if g in BF_PAIRS:
                    _lbf[g] = len(_lbf)
                else:
                    _lfp[g] = len(_lfp)
            # Hold the proto stream until the 2.1 MB of weights has the DMA
            # to itself: outstanding DMAs round-robin at packet granularity,
            # so without this the weight DMAs (which gate qp and thereby all
            # scores) take ~13us instead of ~6. The protos have huge slack -
            # DVE consumes them at ~1.7x their delivery time.
            all_tiles = []
            with tc.tile_wait_until(0.003):
                for g in range(NPG):
                    if g in BF_PAIRS:
                        lg = _lbf[g]
                        T2 = ppoolb.tile(
                            [128, NBLK, PG, D_CTX], bf16, tag="Tb", name="T2b")
                        nc.sync.dma_start(
                            out=T2[:],
                            in_=protobf_v[:, :, lg * PG:(lg + 1) * PG, :],
                        )
                    else:
                        lg = _lfp[g]
                        T2 = ppool.tile(
                            [128, NBLK, PG, D_CTX], f16, tag="T", name="T2")
                        nc.sync.dma_start(
                            out=T2[:],
                            in_=proto_v[:, :, lg * PG:(lg + 1) * PG, :],
                        )
                    all_tiles.append(T2)
            # pooling MACs for chunk c, emitted AFTER chunk c+1's scores:
            # engine queues are in-order, so a DVE diag (which waits on
            # ACT's exp of its chunk) emitted before the next chunk's amr
            # ops would head-of-line-block them.
            def emit_diag_macs(c, chunk_g0, ntiles):
                cp0 = chunk_g0 * PG
                for t in range(ntiles):
                    T2 = all_tiles[chunk_g0 + t]
                    for j in range(PG):
                        p = (chunk_g0 + t) * PG + j
                        q = COLMAP[p]
                        for b in range(NBLK):
                            e_p = expw[c][:, b, q:q + 1]
                            dg = diagp.tile([128, BLK], bf16, tag="dg")
                            if p in DIAG_ON_ACT:
                                nc.scalar.activation(
                                    out=dg[:], in_=eye_sb[:], func=Act.Copy,
                                    scale=e_p,
                                )
                            else:
                                nc.vector.tensor_scalar(
                                    out=dg[:], in0=eye_sb[:], scalar1=e_p,
                                    scalar2=None, op0=Alu.mult,
                                )
                            for h in range(2):
                                nc.tensor.matmul(
                                    U[b][h][:],
                                    dg[:],
                                    T2[:, b, j, h * EH:(h + 1) * EH],
                                    start=(p == 0),
                                    stop=(p == P - 1),
                                    skip_group_check=True,
                                )

            # Within-chunk column order: chunk 0 is permuted (bf16 protos in
            # cols 0-5, fp16 in 6-11) so the fp16 halves can be reduced into
            # one contiguous slice. Softmax is order-invariant per chunk;
            # diag/MAC look e_p up through this map.
            COLMAP = {}
            _g = 0
            for c, ntiles in enumerate(CHUNK_TILES):
                ps_chunk = list(range(_g * PG, (_g + ntiles) * PG))
                if c == 0:
                    ps_chunk = ([p for p in ps_chunk if p in BF_P]
                                + [p for p in ps_chunk if p not in BF_P])
                for qi, p in enumerate(ps_chunk):
                    COLMAP[p] = qi
                _g += ntiles
            nbf0 = sum(1 for p in range(CHUNK_TILES[0] * PG) if p in BF_P)
            nfp0 = CHUNK_TILES[0] * PG - nbf0
            # chunk-0 fp16 score halves: amr against the two PSUM qp halves
            # (skips the ~5us wait for the SBUF qp copies at startup)
            s0h = persist.tile([128, NBLK, nfp0, 2], f32, name="s0h")

            g0 = 0
            chunk_g0s = []
            for c, ntiles in enumerate(CHUNK_TILES):
                cp0, cp1 = g0 * PG, (g0 + ntiles) * PG   # proto range of chunk
                chunk_g0s.append(g0)
                chunk_gs = list(range(g0, g0 + ntiles))
                if c == 0:
                    # fp16 pairs first: they start at PSUM-stop, before the
                    # bf16 qp copy the TT pairs wait on
                    chunk_gs = ([g for g in chunk_gs if g not in BF_PAIRS]
                                + [g for g in chunk_gs if g in BF_PAIRS])
                for g in chunk_gs:
                    T2 = all_tiles[g]
                    for j in range(PG):
                        p = g * PG + j
                        q = COLMAP[p]        # column within this chunk
                        for b in range(NBLK):
                            if p in BF_P:
                                prod = prodp.tile(
                                    [128, D_CTX], bf16, tag="prb", name="prb")
                                nc.vector.tensor_tensor(
                                    out=prod[:], in0=T2[:, b, j, :],
                                    in1=qpb_sb[:, b, :], op=Alu.mult,
                                )
                                nc.scalar.activation(
                                    out=prod[:], in_=prod[:], func=Act.Copy,
                                    accum_out=scores[c][:, b, q:q + 1],
                                )
                            elif c == 0:
                                fq = q - nbf0
                                for h in range(2):
                                    prodh = prodp.tile(
                                        [128, EH], f16, tag="prh", name="prodh")
                                    nc.vector.affine_mul_reduce(
                                        out=prodh[:],
                                        accum_out=s0h[:, b, fq, h:h + 1],
                                        in0=T2[:, b, j, h * EH:(h + 1) * EH],
                                        in1=qp_psums[b][h][:],
                                        scale=1.0,
                                        bias=0.0,
                                    )
                            else:
                                prod = prodp.tile(
                                    [128, D_CTX], f16, tag="pr", name="prod")
                                nc.vector.affine_mul_reduce(
                                    out=prod[:],
                                    accum_out=scores[c][:, b, q:q + 1],
                                    in0=T2[:, b, j, :],
                                    in1=qp_sb[:, b, :],
                                    scale=1.0,
                                    bias=0.0,
                                )

                for b in range(NBLK):
                    s_ch = scores[c][:, b, :]
                    e_ch = expw[c][:, b, :]
                    if c == 0:
                        # combine the fp16 protos' two qp-half partial sums
                        nc.vector.tensor_reduce(
                            out=scores[0][:, b, nbf0:nbf0 + nfp0],
                            in_=s0h[:, b, :, :], axis=X, op=Alu.add,
                        )
                    if c == 0:
                        # fixed exponent frame from chunk 0 (see header)
                        m0 = small.tile([128, 1], f32, tag="m0")
                        nc.vector.tensor_reduce(
                            out=m0[:], in_=s_ch, axis=X, op=Alu.max,
                        )
                        nc.vector.tensor_scalar(
                            negMhat[:, b, :], m0[:], -1.0, -60.0,
                            Alu.mult, Alu.add,
                        )
                        nc.vector.tensor_scalar(
                            clampv[:, b, :], m0[:], 1.0, 140.0,
                            Alu.mult, Alu.add,
                        )
                    else:
                        nc.vector.tensor_scalar(
                            s_ch, s_ch, clampv[:, b, :], None, Alu.min,
                        )
                    nc.scalar.activation(
                        out=e_ch, in_=s_ch, func=Act.Exp,
                        bias=negMhat[:, b, :], scale=1.0,
                        accum_out=zparts[:, b, c:c + 1],
                    )

                if c >= 1:
                    emit_diag_macs(c - 1, chunk_g0s[c - 1], CHUNK_TILES[c - 1])
                g0 += ntiles
            emit_diag_macs(NCH - 1, chunk_g0s[-1], CHUNK_TILES[-1])

            # ---- finalize: ctx = U / Z -> f32 out --------------------------
            for b in range(NBLK):
                zsum = small.tile([128, 1], f32, tag="zs")
                nc.vector.tensor_reduce(
                    out=zsum[:], in_=zparts[:, b, :], axis=X, op=Alu.add,
                )
                rinv = small.tile([128, 1], f32, tag="ri")
                nc.vector.reciprocal(out=rinv[:], in_=zsum[:])
                # h=0 on ACT, h=1 on DVE: the two normalizations run in
                # parallel during the latency-critical tail
                nc.scalar.activation(
                    out=outsb[:, b, 0:EH],
                    in_=U[b][0][:], func=Act.Copy, scale=rinv[:],
                )
                nc.sync.dma_start(
                    out=out_d[b * BLK:(b + 1) * BLK, 0:EH],
                    in_=outsb[:, b, 0:EH],
                )
                nc.vector.tensor_scalar(
                    out=outsb[:, b, EH:D_CTX], in0=U[b][1][:],
                    scalar1=rinv[:], scalar2=None, op0=Alu.mult,
                )
                nc.sync.dma_start(
                    out=out_d[b * BLK:(b + 1) * BLK, EH:D_CTX],
                    in_=outsb[:, b, EH:D_CTX],
                )

    nc.compile()
    return nc


def _get_nc():
    global _NC
    if _NC is None:
        _NC = _build()
    return _NC


def _make_in_maps(sent_vecs, proto_vecs, Wq, bq, Wk):
    import ml_dtypes
    sent = np.asarray(sent_vecs, dtype=np.float32).reshape(TOK, D_SENT)
    sentT = np.ascontiguousarray(sent.T).astype(np.float16)   # [1024, 2048]
    proto = np.asarray(proto_vecs, dtype=np.float32).reshape(TOK, P, D_CTX)
    fp_idx = [2 * g + j for g in range(NPG) if g not in BF_PAIRS
              for j in range(PG)]
    bf_idx = [2 * g + j for g in sorted(BF_PAIRS) for j in range(PG)]
    proto16 = np.ascontiguousarray(proto[:, fp_idx, :].astype(np.float16))
    protobf = np.ascontiguousarray(
        proto[:, bf_idx, :].astype(ml_dtypes.bfloat16))
    wq = np.asarray(Wq, dtype=np.float32)
    bq = np.asarray(bq, dtype=np.float32).reshape(1, D_CTX)
    wk = np.asarray(Wk, dtype=np.float32)
    # fold the projection weights host-side: qp = sent @ W + bp; stage w
    # pre-relayouted to the on-chip [partition, dd, e] layout
    w = (wq @ wk.T).astype(np.float16)
    w = np.ascontiguousarray(w.reshape(DS, 128, D_CTX).transpose(1, 0, 2))
    bp = np.ascontiguousarray((bq @ wk.T).astype(np.float16))
    eye = np.eye(BLK, dtype=np.float32).astype(ml_dtypes.bfloat16)
    in_maps = []
    for i in range(N_CORES):
        sl = slice(i * TPC, (i + 1) * TPC)
        sentT_i = sentT[:, sl].reshape(DS, 128, TPC).transpose(1, 0, 2)
        in_maps.append(
            {
                "sentT": np.ascontiguousarray(sentT_i),
                "proto": np.ascontiguousarray(proto16[sl]),
                "protobf": np.ascontiguousarray(protobf[sl]),
                "w": w,
                "bp": bp,
                "eye": eye,
            }
        )
    return in_maps


def _ensure_ntff_hook():
    """The agent image's antenv lacks axon_hooks; shim it so trace=True
    can capture NTFF profiles via the libaxon ctypes path."""
    try:
        from antenv.axon_hooks import get_axon_ntff_profile_hook  # noqa: F401
        return
    except ImportError:
        pass
    import types

    import antenv
    from trn_agent_boot.trn_boot import _ntff_profile_via_ctypes

    mod = types.ModuleType("antenv.axon_hooks")
    mod._hook = _ntff_profile_via_ctypes("/opt/axon/libaxon_pjrt.so")
    mod.get_axon_ntff_profile_hook = lambda: mod._hook
    mod.set_axon_ntff_profile_hook = lambda h: setattr(mod, "_hook", h)
    sys.modules["antenv.axon_hooks"] = mod
    antenv.axon_hooks = mod


def run(sent_vecs, proto_vecs, Wq, bq, Wk, bk=None, trace=False, **kw):
    """Returns (out[4,512,768] float32, BassKernelResults)."""
    from concourse.bass_utils import run_bass_kernel_spmd

    if trace:
        _ensure_ntff_hook()
    nc = _get_nc()
    in_maps = _make_in_maps(sent_vecs, proto_vecs, Wq, bq, Wk)
    res = run_bass_kernel_spmd(
        nc, in_maps, core_ids=list(range(N_CORES)), trace=trace
    )
    outs = [np.asarray(res.results[i]["out"]) for i in range(N_CORES)]
    full = np.concatenate(outs, axis=0).reshape(B, S, D_CTX).astype(np.float32)
    return full, res


def kernel(sent_vecs, proto_vecs, Wq, bq, Wk, bk=None, **kw):
    out, _ = run(sent_vecs, proto_vecs, Wq, bq, Wk, bk)
    return out


if __name__ == "__main__":
    nc = _get_nc()
    print("build + compile OK")
    from concourse.timeline_sim import TimelineSim
    print("TimelineSim predicted:", TimelineSim(nc).simulate(), "ns")
